# revision 3
# baseline (speedup 1.0000x reference)
"""Trainium2 Bass kernel: attention 'general' score + sequence softmax.

Computes, for full inputs
    hidden [1, 64, 1024], encoder_outputs [2048, 64, 1024], W [1024, 1024]:
    hq = hidden[0] @ W
    energies[i, b] = sum_d hq[b, d] * encoder_outputs[i, b, d]
    out = softmax(energies, axis=0)            # [2048, 64]

Distribution: encoder_outputs sharded along seq (axis 0) across 8 cores;
hidden/W replicated. Per-shard softmax stats (running max + exp-sums) are
combined with one tiny AllGather (log-sum-exp combine), then each core
rescales its local exp tile and writes its [256, 64] output shard.

Per-core layout: shard rows flattened to [16384, 1024]; 128-row tiles map
row t*128+p to partition p (so partition p always holds batch b = p % 64).
A fused DVE tensor_tensor_reduce (mult + add-reduce) produces one energies
column per 128-row tile; ScalarE does exp with a free-axis accumulate.
"""

import sys

import numpy as np

sys.path.insert(0, "/opt/trn_rl_repo")

SEQ_LEN, BATCH, HIDDEN = 2048, 64, 1024
N_CORES = 8
SHARD = SEQ_LEN // N_CORES  # 256 seq positions per core
ROWS = SHARD * BATCH  # 16384 flattened (i, b) rows per core
P = 128  # SBUF partitions
RPT = 4  # 128-row groups per streaming DMA (2 MiB)
NT = ROWS // P  # 128 energy columns per core
NDMA = NT // RPT  # 32 streaming DMAs

_CACHE: dict = {}


def _build():
    from concourse import bacc, mybir, tile

    f32 = mybir.dt.float32
    Alu = mybir.AluOpType
    Act = mybir.ActivationFunctionType

    nc = bacc.Bacc(
        "TRN2", target_bir_lowering=False, debug=False, num_devices=N_CORES
    )
    enc = nc.dram_tensor("enc", [SHARD, BATCH, HIDDEN], f32, kind="ExternalInput")
    hT = nc.dram_tensor("hT", [HIDDEN, BATCH], f32, kind="ExternalInput")
    Wt = nc.dram_tensor("W", [HIDDEN, HIDDEN], f32, kind="ExternalInput")
    out = nc.dram_tensor("out", [SHARD, BATCH], f32, kind="ExternalOutput")

    with tile.TileContext(nc) as tc:
        with (
            tc.tile_pool(name="const", bufs=1) as cpool,
            tc.tile_pool(name="io", bufs=4) as iopool,
            tc.tile_pool(name="scratch", bufs=2) as spool,
            tc.tile_pool(name="psum", bufs=1, space="PSUM") as psum,
            tc.tile_pool(name="dram", bufs=1, space="DRAM") as dram,
        ):
            # ---- hq[b, j] = sum_k hidden[b, k] W[k, j] on the PE ----
            # lhsT = hiddenT chunk [k=128, b=64], rhs = W chunk [k=128, j<=512].
            w_sb = cpool.tile([P, 8, HIDDEN], f32)
            w_src = Wt.ap().rearrange("(c p) j -> p c j", p=P)
            for c in range(8):
                nc.sync.dma_start(w_sb[:, c, :], w_src[:, c, :])
            hT_sb = cpool.tile([P, 8, BATCH], f32)
            nc.sync.dma_start(hT_sb[:], hT.ap().rearrange("(c p) b -> p c b", p=P))

            hq_ps = psum.tile([BATCH, HIDDEN], f32)
            for h in range(2):
                for c in range(8):
                    nc.tensor.matmul(
                        hq_ps[:, h * 512 : (h + 1) * 512],
                        hT_sb[:, c, :],
                        w_sb[:, c, h * 512 : (h + 1) * 512],
                        start=(c == 0),
                        stop=(c == 7),
                    )
            # hq duplicated onto both partition halves: hq2[p] = hq[p % 64]
            hq2 = cpool.tile([P, HIDDEN], f32)
            nc.scalar.copy(hq2[0:BATCH, :], hq_ps[:])
            nc.sync.dma_start(hq2[BATCH:P, :], hq2[0:BATCH, :])

            # Load the ScalarE Exp table while the stream runs.
            warm = cpool.tile([P, 1], f32)
            nc.gpsimd.memset(warm[:], 0.0)
            nc.scalar.activation(warm[:], warm[:], Act.Exp)

            # ---- stream encoder shard, fused multiply + reduce ----
            energies = cpool.tile([P, NT], f32)
            blocks = enc.ap().rearrange(
                "(td r q) b d -> td (q b) r d", td=NDMA, r=RPT, q=2
            )
            for td in range(NDMA):
                et = iopool.tile([P, RPT, HIDDEN], f32, tag="enc")
                nc.sync.dma_start(et[:], blocks[td])
                for r in range(RPT):
                    t = td * RPT + r
                    prod = spool.tile([P, HIDDEN], f32, tag="prod")
                    nc.vector.scalar_tensor_tensor(
                        out=prod[:],
                        in0=et[:, r, :],
                        scalar=1.0,
                        in1=hq2[:],
                        op0=Alu.mult,
                        op1=Alu.mult,
                        accum_out=energies[:, t : t + 1],
                    )

            # ---- local softmax stats (per partition = per (i-parity, b) group) ----
            m128 = cpool.tile([P, 1], f32)
            nc.vector.tensor_reduce(
                m128[:], energies[:], axis=mybir.AxisListType.X, op=Alu.max
            )
            nm128 = cpool.tile([P, 1], f32)
            nc.vector.tensor_scalar_mul(nm128[:], m128[:], -1.0)
            pexp = cpool.tile([P, NT], f32)
            s128 = cpool.tile([P, 1], f32)
            nc.scalar.activation(
                pexp[:], energies[:], Act.Exp, bias=nm128[:], accum_out=s128[:]
            )

            # ---- one AllGather of (max, sum) stats; log-sum-exp combine ----
            st = cpool.tile([P, 2], f32)
            nc.vector.tensor_copy(st[:, 0:1], m128[:])
            nc.vector.tensor_copy(st[:, 1:2], s128[:])
            cc_in = dram.tile([P, 2], f32)
            cc_out = dram.tile([N_CORES, P, 2], f32, addr_space="Shared")
            nc.sync.dma_start(cc_in[:], st[:])
            nc.gpsimd.collective_compute(
                "AllGather",
                Alu.bypass,
                replica_groups=[list(range(N_CORES))],
                ins=[cc_in[:].opt()],
                outs=[cc_out[:].opt()],
            )
            # g[b, core, parity, stat]
            g = cpool.tile([BATCH, N_CORES, 2, 2], f32)
            nc.sync.dma_start(
                g[:], cc_out.rearrange("c (q b) j -> b c q j", q=2)
            )
            M64 = cpool.tile([BATCH, 1], f32)
            nc.vector.tensor_reduce(
                M64[:], g[:, :, :, 0], axis=mybir.AxisListType.XY, op=Alu.max
            )
            nM64 = cpool.tile([BATCH, 1], f32)
            nc.vector.tensor_scalar_mul(nM64[:], M64[:], -1.0)
            wexp = cpool.tile([BATCH, N_CORES, 2], f32)
            nc.scalar.activation(wexp[:], g[:, :, :, 0], Act.Exp, bias=nM64[:])
            ws = cpool.tile([BATCH, N_CORES, 2], f32)
            nc.vector.tensor_mul(ws[:], wexp[:], g[:, :, :, 1])
            S64 = cpool.tile([BATCH, 1], f32)
            nc.vector.tensor_reduce(
                S64[:], ws[:], axis=mybir.AxisListType.XY, op=Alu.add
            )
            rS = cpool.tile([BATCH, 1], f32)
            nc.vector.reciprocal(rS[:], S64[:])

            # broadcast (M, 1/S) to both partition halves
            pk = cpool.tile([BATCH, 2], f32)
            nc.vector.tensor_copy(pk[:, 0:1], M64[:])
            nc.vector.tensor_copy(pk[:, 1:2], rS[:])
            gb = cpool.tile([P, 2], f32)
            nc.vector.tensor_copy(gb[0:BATCH, :], pk[:])
            nc.sync.dma_start(gb[BATCH:P, :], pk[:])

            # out = pexp * exp(m128 - M) / S
            nMb = cpool.tile([P, 1], f32)
            nc.vector.tensor_scalar_mul(nMb[:], gb[:, 0:1], -1.0)
            f_scale = cpool.tile([P, 1], f32)
            nc.scalar.activation(f_scale[:], m128[:], Act.Exp, bias=nMb[:])
            nc.vector.tensor_mul(f_scale[:], f_scale[:], gb[:, 1:2])
            o_sb = cpool.tile([P, NT], f32)
            nc.vector.tensor_scalar(
                o_sb[:], pexp[:], f_scale[:], None, op0=Alu.mult
            )
            out_view = out.ap().rearrange("i b -> (i b)").rearrange(
                "(c p) -> p c", p=P
            )
            nc.sync.dma_start(out_view, o_sb[:])

    nc.compile()
    return nc


def _get_nc():
    if "nc" not in _CACHE:
        _CACHE["nc"] = _build()
    return _CACHE["nc"]


def _in_maps(hidden, encoder_outputs, W):
    hidden = np.asarray(hidden, dtype=np.float32)
    encoder_outputs = np.ascontiguousarray(encoder_outputs, dtype=np.float32)
    W = np.ascontiguousarray(W, dtype=np.float32)
    hT = np.ascontiguousarray(hidden[0].T)  # [1024, 64]
    return [
        {
            "enc": np.ascontiguousarray(
                encoder_outputs[c * SHARD : (c + 1) * SHARD]
            ),
            "hT": hT,
            "W": W,
        }
        for c in range(N_CORES)
    ]


def _gather(results):
    return np.concatenate(
        [np.asarray(results[c]["out"]) for c in range(N_CORES)], axis=0
    )


def kernel(hidden, encoder_outputs, W):
    from concourse import bass_utils

    nc = _get_nc()
    res = bass_utils.run_bass_kernel_spmd(
        nc, _in_maps(hidden, encoder_outputs, W), core_ids=list(range(N_CORES))
    )
    return _gather(res.results)


def run_traced(hidden, encoder_outputs, W, **trace_kwargs):
    """Run with neuron-profile tracing; returns (output, BassKernelResults)."""
    from concourse import bass_utils

    nc = _get_nc()
    res = bass_utils.run_bass_kernel_spmd(
        nc,
        _in_maps(hidden, encoder_outputs, W),
        core_ids=list(range(N_CORES)),
        trace=True,
        **trace_kwargs,
    )
    return _gather(res.results), res


# revision 4
# speedup vs baseline: 1.3093x; 1.3093x over previous
"""Trainium2 Bass kernel: attention 'general' score + sequence softmax.

Computes, for full inputs
    hidden [1, 64, 1024], encoder_outputs [2048, 64, 1024], W [1024, 1024]:
    hq = hidden[0] @ W
    energies[i, b] = sum_d hq[b, d] * encoder_outputs[i, b, d]
    out = softmax(energies, axis=0)            # [2048, 64]

Distribution: encoder_outputs sharded along seq (axis 0) across 8 cores;
hidden/W replicated. Per-shard softmax stats (max + exp-sum per partition)
are combined with one tiny AllGather (log-sum-exp combine), then each core
rescales its local exp tile and writes its output shard.

Per-core layout: shard rows flattened to [16384, 1024]; row t*128 + p lives
on partition p (partition p always holds batch b = p % 64). The host
pre-packs every input into partition-major order so each DMA descriptor
moves a 16-32 KiB contiguous run. A fused DVE scalar_tensor_tensor
(mult + sum-reduce) produces one energies column per 128-row group;
ScalarE does exp with a free-axis accumulate. The output shard is written
partition-major [128, 128] and transposed back on the host.
"""

import sys

import numpy as np

sys.path.insert(0, "/opt/trn_rl_repo")

SEQ_LEN, BATCH, HIDDEN = 2048, 64, 1024
N_CORES = 8
SHARD = SEQ_LEN // N_CORES  # 256 seq positions per core
ROWS = SHARD * BATCH  # 16384 flattened (i, b) rows per core
P = 128  # SBUF partitions
RPT = 4  # 128-row groups per streaming DMA (2 MiB)
NT = ROWS // P  # 128 energy columns per core
NDMA = NT // RPT  # 32 streaming DMAs

_CACHE: dict = {}


def _build():
    from concourse import bacc, mybir, tile

    f32 = mybir.dt.float32
    Alu = mybir.AluOpType
    Act = mybir.ActivationFunctionType

    nc = bacc.Bacc(
        "TRN2", target_bir_lowering=False, debug=False, num_devices=N_CORES
    )
    # All inputs host-packed partition-major (see _in_maps).
    enc = nc.dram_tensor("enc", [NDMA, P, RPT * HIDDEN], f32, kind="ExternalInput")
    hT2 = nc.dram_tensor("hT2", [P, 8, P], f32, kind="ExternalInput")
    Wt = nc.dram_tensor("W", [P, 8, HIDDEN], f32, kind="ExternalInput")
    out = nc.dram_tensor("out", [P, NT], f32, kind="ExternalOutput")

    with tile.TileContext(nc) as tc:
        with (
            tc.tile_pool(name="const", bufs=1) as cpool,
            tc.tile_pool(name="io", bufs=4) as iopool,
            tc.tile_pool(name="scratch", bufs=2) as spool,
            tc.tile_pool(name="psum", bufs=1, space="PSUM") as psum,
            tc.tile_pool(name="dram", bufs=1, space="DRAM") as dram,
        ):
            # ---- hq2[p, j] = sum_k hidden[p % 64, k] W[k, j] on the PE ----
            # lhsT = duplicated-hidden chunk [k=128, m=128], rhs = W chunk.
            w_sb = cpool.tile([P, 8, HIDDEN], f32)
            nc.sync.dma_start(w_sb[:], Wt.ap())
            h_sb = cpool.tile([P, 8, P], f32)
            nc.sync.dma_start(h_sb[:], hT2.ap())

            hq_ps = psum.tile([P, HIDDEN], f32)
            for h in range(2):
                for c in range(8):
                    nc.tensor.matmul(
                        hq_ps[:, h * 512 : (h + 1) * 512],
                        h_sb[:, c, :],
                        w_sb[:, c, h * 512 : (h + 1) * 512],
                        start=(c == 0),
                        stop=(c == 7),
                    )
            hq2 = cpool.tile([P, HIDDEN], f32)
            nc.scalar.copy(hq2[:], hq_ps[:])

            # Load the ScalarE Exp table while the stream runs.
            warm = cpool.tile([P, 1], f32)
            nc.gpsimd.memset(warm[:], 0.0)
            nc.scalar.activation(warm[:], warm[:], Act.Exp)

            # ---- stream encoder shard, fused multiply + sum-reduce ----
            energies = cpool.tile([P, NT], f32)
            for td in range(NDMA):
                et = iopool.tile([P, RPT * HIDDEN], f32, tag="enc")
                nc.sync.dma_start(et[:], enc.ap()[td])
                for r in range(RPT):
                    t = td * RPT + r
                    prod = spool.tile([P, HIDDEN], f32, tag="prod")
                    nc.vector.scalar_tensor_tensor(
                        out=prod[:],
                        in0=et[:, r * HIDDEN : (r + 1) * HIDDEN],
                        scalar=1.0,
                        in1=hq2[:],
                        op0=Alu.mult,
                        op1=Alu.mult,
                        accum_out=energies[:, t : t + 1],
                    )

            # ---- local softmax stats (per partition = per (i-parity, b)) ----
            m128 = cpool.tile([P, 1], f32)
            nc.vector.tensor_reduce(
                m128[:], energies[:], axis=mybir.AxisListType.X, op=Alu.max
            )
            nm128 = cpool.tile([P, 1], f32)
            nc.vector.tensor_scalar_mul(nm128[:], m128[:], -1.0)
            pexp = cpool.tile([P, NT], f32)
            s128 = cpool.tile([P, 1], f32)
            nc.scalar.activation(
                pexp[:], energies[:], Act.Exp, bias=nm128[:], accum_out=s128[:]
            )

            # ---- one AllGather of (max, sum) stats; log-sum-exp combine ----
            st = cpool.tile([P, 2], f32)
            nc.vector.tensor_copy(st[:, 0:1], m128[:])
            nc.vector.tensor_copy(st[:, 1:2], s128[:])
            cc_in = dram.tile([P, 2], f32)
            cc_out = dram.tile([N_CORES, P, 2], f32, addr_space="Shared")
            nc.sync.dma_start(cc_in[:], st[:])
            nc.gpsimd.collective_compute(
                "AllGather",
                Alu.bypass,
                replica_groups=[list(range(N_CORES))],
                ins=[cc_in[:].opt()],
                outs=[cc_out[:].opt()],
            )
            # g[b, core, parity, stat]
            g = cpool.tile([BATCH, N_CORES, 2, 2], f32)
            nc.sync.dma_start(
                g[:], cc_out.rearrange("c (q b) j -> b c q j", q=2)
            )
            M64 = cpool.tile([BATCH, 1], f32)
            nc.vector.tensor_reduce(
                M64[:], g[:, :, :, 0], axis=mybir.AxisListType.XY, op=Alu.max
            )
            nM64 = cpool.tile([BATCH, 1], f32)
            nc.vector.tensor_scalar_mul(nM64[:], M64[:], -1.0)
            wexp = cpool.tile([BATCH, N_CORES, 2], f32)
            nc.scalar.activation(wexp[:], g[:, :, :, 0], Act.Exp, bias=nM64[:])
            ws = cpool.tile([BATCH, N_CORES, 2], f32)
            nc.vector.tensor_mul(ws[:], wexp[:], g[:, :, :, 1])
            S64 = cpool.tile([BATCH, 1], f32)
            nc.vector.tensor_reduce(
                S64[:], ws[:], axis=mybir.AxisListType.XY, op=Alu.add
            )
            rS = cpool.tile([BATCH, 1], f32)
            nc.vector.reciprocal(rS[:], S64[:])

            # broadcast (M, 1/S) to both partition halves
            pk = cpool.tile([BATCH, 2], f32)
            nc.vector.tensor_copy(pk[:, 0:1], M64[:])
            nc.vector.tensor_copy(pk[:, 1:2], rS[:])
            gb = cpool.tile([P, 2], f32)
            nc.vector.tensor_copy(gb[0:BATCH, :], pk[:])
            nc.sync.dma_start(gb[BATCH:P, :], pk[:])

            # out = pexp * exp(m128 - M) / S   (partition-major; host transposes)
            nMb = cpool.tile([P, 1], f32)
            nc.vector.tensor_scalar_mul(nMb[:], gb[:, 0:1], -1.0)
            f_scale = cpool.tile([P, 1], f32)
            nc.scalar.activation(f_scale[:], m128[:], Act.Exp, bias=nMb[:])
            nc.vector.tensor_mul(f_scale[:], f_scale[:], gb[:, 1:2])
            o_sb = cpool.tile([P, NT], f32)
            nc.vector.tensor_scalar(
                o_sb[:], pexp[:], f_scale[:], None, op0=Alu.mult
            )
            nc.sync.dma_start(out.ap(), o_sb[:])

    nc.compile()
    return nc


def _get_nc():
    if "nc" not in _CACHE:
        _CACHE["nc"] = _build()
    return _CACHE["nc"]


def _in_maps(hidden, encoder_outputs, W):
    hidden = np.asarray(hidden, dtype=np.float32)
    encoder_outputs = np.asarray(encoder_outputs, dtype=np.float32)
    W = np.asarray(W, dtype=np.float32)

    # W_packed[p, c, j] = W[c*128 + p, j]
    w_packed = np.ascontiguousarray(
        W.reshape(8, P, HIDDEN).transpose(1, 0, 2)
    )
    # hT2[p, c, m] = hidden[0][m % 64, c*128 + p]
    h2 = np.concatenate([hidden[0], hidden[0]], axis=0)  # [128, 1024]
    hT2 = np.ascontiguousarray(h2.T.reshape(8, P, P).transpose(1, 0, 2))

    maps = []
    for c in range(N_CORES):
        shard = encoder_outputs[c * SHARD : (c + 1) * SHARD]
        # row t*128 + p  ->  enc_packed[t // RPT, p, (t % RPT)*1024 : ...]
        packed = np.ascontiguousarray(
            shard.reshape(NDMA, RPT, P, HIDDEN).transpose(0, 2, 1, 3)
        ).reshape(NDMA, P, RPT * HIDDEN)
        maps.append({"enc": packed, "hT2": hT2, "W": w_packed})
    return maps


def _gather(results):
    shards = []
    for c in range(N_CORES):
        raw = np.asarray(results[c]["out"])  # [128 p, 128 t]
        shards.append(np.ascontiguousarray(raw.T).reshape(SHARD, BATCH))
    return np.concatenate(shards, axis=0)


def kernel(hidden, encoder_outputs, W):
    from concourse import bass_utils

    nc = _get_nc()
    res = bass_utils.run_bass_kernel_spmd(
        nc, _in_maps(hidden, encoder_outputs, W), core_ids=list(range(N_CORES))
    )
    return _gather(res.results)


def run_traced(hidden, encoder_outputs, W, **trace_kwargs):
    """Run with neuron-profile tracing; returns (output, BassKernelResults)."""
    from concourse import bass_utils

    nc = _get_nc()
    res = bass_utils.run_bass_kernel_spmd(
        nc,
        _in_maps(hidden, encoder_outputs, W),
        core_ids=list(range(N_CORES)),
        trace=True,
        **trace_kwargs,
    )
    return _gather(res.results), res


# revision 6
# speedup vs baseline: 1.3298x; 1.0157x over previous
"""Trainium2 Bass kernel: attention 'general' score + sequence softmax.

Computes, for full inputs
    hidden [1, 64, 1024], encoder_outputs [2048, 64, 1024], W [1024, 1024]:
    hq = hidden[0] @ W
    energies[i, b] = sum_d hq[b, d] * encoder_outputs[i, b, d]
    out = softmax(energies, axis=0)            # [2048, 64]

Distribution: encoder_outputs sharded along seq (axis 0) across 8 cores;
hidden/W replicated. Per-shard softmax stats (max + exp-sum per partition)
are combined with one tiny AllGather (log-sum-exp combine), then each core
rescales its local exp tile and writes its output shard.

Per-core layout: shard rows flattened to [16384, 1024]; row t*128 + p lives
on partition p (partition p always holds batch b = p % 64). The host
pre-packs every input into partition-major order so each DMA descriptor
moves a 16-32 KiB contiguous run. A fused DVE scalar_tensor_tensor
(mult + sum-reduce) produces one energies column per 128-row group;
ScalarE does exp with a free-axis accumulate. The output shard is written
partition-major [128, 128] and transposed back on the host.
"""

import sys

import numpy as np

sys.path.insert(0, "/opt/trn_rl_repo")

SEQ_LEN, BATCH, HIDDEN = 2048, 64, 1024
N_CORES = 8
SHARD = SEQ_LEN // N_CORES  # 256 seq positions per core
ROWS = SHARD * BATCH  # 16384 flattened (i, b) rows per core
P = 128  # SBUF partitions
RPT = 4  # 128-row groups per streaming DMA (2 MiB)
NT = ROWS // P  # 128 energy columns per core
NDMA = NT // RPT  # 32 streaming DMAs

_CACHE: dict = {}


def _build():
    from concourse import bacc, mybir, tile

    f32 = mybir.dt.float32
    Alu = mybir.AluOpType
    Act = mybir.ActivationFunctionType

    nc = bacc.Bacc(
        "TRN2", target_bir_lowering=False, debug=False, num_devices=N_CORES
    )
    # All inputs host-packed partition-major (see _in_maps).
    enc = nc.dram_tensor("enc", [NDMA, P, RPT * HIDDEN], f32, kind="ExternalInput")
    hT2 = nc.dram_tensor("hT2", [P, 8, P], f32, kind="ExternalInput")
    Wt = nc.dram_tensor("W", [P, 8, HIDDEN], f32, kind="ExternalInput")
    out = nc.dram_tensor("out", [P, NT], f32, kind="ExternalOutput")

    with tile.TileContext(nc) as tc:
        with (
            tc.tile_pool(name="const", bufs=1) as cpool,
            tc.tile_pool(name="io", bufs=7) as iopool,
            tc.tile_pool(name="scratch", bufs=2) as spool,
            tc.tile_pool(name="psum", bufs=1, space="PSUM") as psum,
            tc.tile_pool(name="dram", bufs=1, space="DRAM") as dram,
        ):
            # ---- hq2[p, j] = sum_k hidden[p % 64, k] W[k, j] on the PE ----
            # lhsT = duplicated-hidden chunk [k=128, m=128], rhs = W chunk.
            h_sb = cpool.tile([P, 8, P], f32)
            nc.sync.dma_start(h_sb[:], hT2.ap())
            w_sb = cpool.tile([P, 8, HIDDEN], f32)
            hq_ps = psum.tile([P, HIDDEN], f32)
            # Per-chunk W DMA so matmul c pipelines behind chunk c's load.
            for c in range(8):
                nc.sync.dma_start(w_sb[:, c, :], Wt.ap()[:, c, :])
                for h in range(2):
                    nc.tensor.matmul(
                        hq_ps[:, h * 512 : (h + 1) * 512],
                        h_sb[:, c, :],
                        w_sb[:, c, h * 512 : (h + 1) * 512],
                        start=(c == 0),
                        stop=(c == 7),
                    )
            hq2 = cpool.tile([P, HIDDEN], f32)
            nc.scalar.copy(hq2[:], hq_ps[:])

            # Load the ScalarE Exp table while the stream runs.
            warm = cpool.tile([P, 2], f32)
            nc.gpsimd.memset(warm[:], 0.0)
            nc.scalar.activation(warm[:, 0:1], warm[:, 0:1], Act.Exp)

            # Warm-up collective: absorbs the all-core start barrier and
            # ncfw setup so the real AllGather at the tail is cheap.
            cc_warm_in = dram.tile([P, 2], f32)
            cc_warm_out = dram.tile([N_CORES, P, 2], f32, addr_space="Shared")
            nc.sync.dma_start(cc_warm_in[:], warm[:])
            nc.gpsimd.collective_compute(
                "AllGather",
                Alu.bypass,
                replica_groups=[list(range(N_CORES))],
                ins=[cc_warm_in[:].opt()],
                outs=[cc_warm_out[:].opt()],
            )

            # ---- stream encoder shard, fused multiply + sum-reduce ----
            energies = cpool.tile([P, NT], f32)
            for td in range(NDMA):
                et = iopool.tile([P, RPT * HIDDEN], f32, tag="enc")
                nc.sync.dma_start(et[:], enc.ap()[td])
                for r in range(RPT):
                    t = td * RPT + r
                    prod = spool.tile([P, HIDDEN], f32, tag="prod")
                    nc.vector.scalar_tensor_tensor(
                        out=prod[:],
                        in0=et[:, r * HIDDEN : (r + 1) * HIDDEN],
                        scalar=1.0,
                        in1=hq2[:],
                        op0=Alu.mult,
                        op1=Alu.mult,
                        accum_out=energies[:, t : t + 1],
                    )

            # ---- local softmax stats (per partition = per (i-parity, b)) ----
            m128 = cpool.tile([P, 1], f32)
            nc.vector.tensor_reduce(
                m128[:], energies[:], axis=mybir.AxisListType.X, op=Alu.max
            )
            nm128 = cpool.tile([P, 1], f32)
            nc.vector.tensor_scalar_mul(nm128[:], m128[:], -1.0)
            pexp = cpool.tile([P, NT], f32)
            s128 = cpool.tile([P, 1], f32)
            nc.scalar.activation(
                pexp[:], energies[:], Act.Exp, bias=nm128[:], accum_out=s128[:]
            )

            # ---- one AllGather of (max, sum) stats; log-sum-exp combine ----
            st = cpool.tile([P, 2], f32)
            nc.vector.tensor_copy(st[:, 0:1], m128[:])
            nc.vector.tensor_copy(st[:, 1:2], s128[:])
            cc_in = dram.tile([P, 2], f32)
            cc_out = dram.tile([N_CORES, P, 2], f32, addr_space="Shared")
            nc.sync.dma_start(cc_in[:], st[:])
            nc.gpsimd.collective_compute(
                "AllGather",
                Alu.bypass,
                replica_groups=[list(range(N_CORES))],
                ins=[cc_in[:].opt()],
                outs=[cc_out[:].opt()],
            )
            # g[b, core, parity, stat]
            g = cpool.tile([BATCH, N_CORES, 2, 2], f32)
            nc.sync.dma_start(
                g[:], cc_out.rearrange("c (q b) j -> b c q j", q=2)
            )
            M64 = cpool.tile([BATCH, 1], f32)
            nc.vector.tensor_reduce(
                M64[:], g[:, :, :, 0], axis=mybir.AxisListType.XY, op=Alu.max
            )
            nM64 = cpool.tile([BATCH, 1], f32)
            nc.vector.tensor_scalar_mul(nM64[:], M64[:], -1.0)
            wexp = cpool.tile([BATCH, N_CORES, 2], f32)
            nc.scalar.activation(wexp[:], g[:, :, :, 0], Act.Exp, bias=nM64[:])
            ws = cpool.tile([BATCH, N_CORES, 2], f32)
            nc.vector.tensor_mul(ws[:], wexp[:], g[:, :, :, 1])
            S64 = cpool.tile([BATCH, 1], f32)
            nc.vector.tensor_reduce(
                S64[:], ws[:], axis=mybir.AxisListType.XY, op=Alu.add
            )
            rS = cpool.tile([BATCH, 1], f32)
            nc.vector.reciprocal(rS[:], S64[:])

            # broadcast (M, 1/S) to both partition halves
            pk = cpool.tile([BATCH, 2], f32)
            nc.vector.tensor_copy(pk[:, 0:1], M64[:])
            nc.vector.tensor_copy(pk[:, 1:2], rS[:])
            gb = cpool.tile([P, 2], f32)
            nc.vector.tensor_copy(gb[0:BATCH, :], pk[:])
            nc.sync.dma_start(gb[BATCH:P, :], pk[:])

            # out = pexp * exp(m128 - M) / S   (partition-major; host transposes)
            nMb = cpool.tile([P, 1], f32)
            nc.vector.tensor_scalar_mul(nMb[:], gb[:, 0:1], -1.0)
            f_scale = cpool.tile([P, 1], f32)
            nc.scalar.activation(f_scale[:], m128[:], Act.Exp, bias=nMb[:])
            nc.vector.tensor_mul(f_scale[:], f_scale[:], gb[:, 1:2])
            o_sb = cpool.tile([P, NT], f32)
            nc.vector.tensor_scalar(
                o_sb[:], pexp[:], f_scale[:], None, op0=Alu.mult
            )
            nc.sync.dma_start(out.ap(), o_sb[:])

    nc.compile()
    return nc


def _get_nc():
    if "nc" not in _CACHE:
        _CACHE["nc"] = _build()
    return _CACHE["nc"]


def _in_maps(hidden, encoder_outputs, W):
    hidden = np.asarray(hidden, dtype=np.float32)
    encoder_outputs = np.asarray(encoder_outputs, dtype=np.float32)
    W = np.asarray(W, dtype=np.float32)

    # W_packed[p, c, j] = W[c*128 + p, j]
    w_packed = np.ascontiguousarray(
        W.reshape(8, P, HIDDEN).transpose(1, 0, 2)
    )
    # hT2[p, c, m] = hidden[0][m % 64, c*128 + p]
    h2 = np.concatenate([hidden[0], hidden[0]], axis=0)  # [128, 1024]
    hT2 = np.ascontiguousarray(h2.T.reshape(8, P, P).transpose(1, 0, 2))

    maps = []
    for c in range(N_CORES):
        shard = encoder_outputs[c * SHARD : (c + 1) * SHARD]
        # row t*128 + p  ->  enc_packed[t // RPT, p, (t % RPT)*1024 : ...]
        packed = np.ascontiguousarray(
            shard.reshape(NDMA, RPT, P, HIDDEN).transpose(0, 2, 1, 3)
        ).reshape(NDMA, P, RPT * HIDDEN)
        maps.append({"enc": packed, "hT2": hT2, "W": w_packed})
    return maps


def _gather(results):
    shards = []
    for c in range(N_CORES):
        raw = np.asarray(results[c]["out"])  # [128 p, 128 t]
        shards.append(np.ascontiguousarray(raw.T).reshape(SHARD, BATCH))
    return np.concatenate(shards, axis=0)


def kernel(hidden, encoder_outputs, W):
    from concourse import bass_utils

    nc = _get_nc()
    res = bass_utils.run_bass_kernel_spmd(
        nc, _in_maps(hidden, encoder_outputs, W), core_ids=list(range(N_CORES))
    )
    return _gather(res.results)


def run_traced(hidden, encoder_outputs, W, **trace_kwargs):
    """Run with neuron-profile tracing; returns (output, BassKernelResults)."""
    from concourse import bass_utils

    nc = _get_nc()
    res = bass_utils.run_bass_kernel_spmd(
        nc,
        _in_maps(hidden, encoder_outputs, W),
        core_ids=list(range(N_CORES)),
        trace=True,
        **trace_kwargs,
    )
    return _gather(res.results), res
